# revision 15
# baseline (speedup 1.0000x reference)
"""2-layer GCN (GCNConv -> ReLU -> GCNConv) on 8 TRN2 NeuronCores.

Strategy (sliced-ELLPACK, node sharding, bf16 slots):
  GCN algebra: out = D^-1/2 (A+I) D^-1/2 (relu(D^-1/2 (A+I) D^-1/2 x W1 + b1)) W2 + b2.
  Normalization is separable (norm_e = dinv[row]*dinv[col]) and aggregation
  linear, so each layer is: per-node scale -> unweighted neighbor-sum ->
  per-node scale -> dense matmul. Self-loop contributions are dense adds.

  Host (pure index layout, no model math): sort nodes by in-degree, pack each
  node's incoming edges into padded slot rows (sliced ELLPACK, slices of 128
  nodes, DP-chosen groups of slices sharing a width). Slots carry the *input*
  features x[row] (bf16) and the integer structural count deg[row] (bf16,
  exact). Device computes all FP model math: rsqrt on the scalar engine,
  per-slot scaling + segmented reductions on DVE (bf16 2x mode), the
  2->16->1 MLP as a scalar_tensor_tensor chain with the per-j feature scale
  folded into the scalar-engine relu via a device-computed b/a ratio.

  Two launches: A computes layer 1 + q = dinv * (h @ W2); host re-shards q
  into the same slot layout (index gather only); B reduces q-slots and
  finishes layer 2. Nodes sharded round-robin by 128-node slice; edge slots
  live with their target node.
"""
import numpy as np
import ml_dtypes

BF16 = ml_dtypes.bfloat16
P = 128
N_CORES = 8
NSL = 256            # local slices per core -> 2048 global slices
NRANKS = 2048 * P    # padded rank space (262144 >= 250000)
LAM = 60             # DP penalty (columns) per extra width-group
N_CHUNKS = 3         # slot-array DMA chunks

TRACE = False
_cache = {}


def _install_ntff_shim():
    import contextlib, ctypes, sys, types
    if "antenv.axon_hooks" in sys.modules:
        return
    try:
        lib = ctypes.CDLL("/opt/axon/libaxon_pjrt.so")
        if not hasattr(lib, "axon_start_nrt_profile"):
            return
        lib.axon_start_nrt_profile.argtypes = [ctypes.POINTER(ctypes.c_int64), ctypes.c_size_t]
        lib.axon_start_nrt_profile.restype = ctypes.c_int64
        lib.axon_stop_nrt_profile.argtypes = [ctypes.c_char_p]
        lib.axon_stop_nrt_profile.restype = ctypes.c_int64
    except OSError:
        return

    @contextlib.contextmanager
    def _hook(output_dir, device_ids):
        import jax
        jax.devices()
        if device_ids:
            ids = (ctypes.c_int64 * len(device_ids))(*device_ids)
            rc = lib.axon_start_nrt_profile(ids, len(device_ids))
        else:
            rc = lib.axon_start_nrt_profile(None, 0)
        if rc != 0:
            raise RuntimeError(f"axon_start_nrt_profile rc={rc}")
        try:
            yield
        finally:
            lib.axon_stop_nrt_profile(str(output_dir).encode())

    mod = types.ModuleType("antenv.axon_hooks")
    mod.get_axon_ntff_profile_hook = lambda: _hook
    mod.set_axon_ntff_profile_hook = lambda h: None
    sys.modules["antenv.axon_hooks"] = mod


def _plan_groups(W_l):
    """DP partition of local slices into contiguous width-groups.
    Boundaries restricted to even slice indices so every group block has an
    even column count (keeps bf16 sub-blocks 4B-aligned for DVE 2x mode)."""
    nsl = len(W_l)
    INF = 1 << 60
    best = np.full(nsl + 1, INF, np.int64)
    best[0] = 0
    ch = np.zeros(nsl + 1, np.int64)
    for e in range(2, nsl + 1, 2):
        s = np.arange(0, e, 2)
        c = best[s] + (e - s) * W_l[s] + LAM
        i = int(np.argmin(c))
        best[e] = c[i]
        ch[e] = 2 * i
    bnds = []
    e = nsl
    while e > 0:
        s = int(ch[e])
        bnds.append((s, e))
        e = s
    bnds.reverse()
    groups = []
    off = 0
    for s, e in bnds:
        w = int(W_l[s])
        groups.append((s, e, w, off))   # (l0, l1, width, column offset in dgs)
        off += (e - s) * w
    return groups, off                  # TOT = off


def _chunk_groups(groups):
    """Split groups into DMA chunks with graded sizes (small first chunk so
    compute starts as early as possible while later transfers overlap)."""
    tot = sum((e - s) * w for s, e, w, _ in groups)
    fracs = [0.22, 0.58, 1.01]
    chunks = []
    cur = []
    acc = 0
    for g in groups:
        s, e, w, _ = g
        cur.append(g)
        acc += (e - s) * w
        if len(chunks) < len(fracs) - 1 and acc >= fracs[len(chunks)] * tot:
            chunks.append(cur)
            cur = []
    if cur:
        chunks.append(cur)
    return [c for c in chunks if c]


def _build_programs(groups, TOT):
    import concourse.bass as bass
    import concourse.bacc as bacc
    import concourse.tile as tile
    import concourse.mybir as mybir

    f32 = mybir.dt.float32
    bf16 = mybir.dt.bfloat16
    AF = mybir.ActivationFunctionType
    ALU = mybir.AluOpType
    X = mybir.AxisListType.X
    chunks = _chunk_groups(groups)

    # ---------------- program A ----------------
    ncA = bacc.Bacc("TRN2", target_bir_lowering=False, debug=False, num_devices=N_CORES)
    x01s = ncA.dram_tensor("x01s", [P, 2 * TOT], bf16, kind="ExternalInput")
    dgs = ncA.dram_tensor("dgs", [P, TOT], bf16, kind="ExternalInput")
    tb = ncA.dram_tensor("tb", [P, 768], bf16, kind="ExternalInput")    # xt01 | dgt
    wb = ncA.dram_tensor("wb", [P, 65], f32, kind="ExternalInput")
    sOut = ncA.dram_tensor("sOut", [P, NSL], f32, kind="ExternalOutput")

    with tile.TileContext(ncA) as tc:
        with tc.tile_pool(name="slots", bufs=2) as pool, \
             tc.tile_pool(name="persist", bufs=1) as pp:
            tbt = pp.tile([P, 768], bf16)
            ncA.scalar.dma_start(tbt[:], tb.ap())
            wbt = pp.tile([P, 65], f32)
            ncA.scalar.dma_start(wbt[:], wb.ap())
            agg = pp.tile([P, 512], bf16)

            # node-table prep: dinv, dinv^2, weight ratios
            dinv = pp.tile([P, NSL], bf16)
            ncA.scalar.activation(dinv[:], tbt[:, 512:768], AF.Abs_reciprocal_sqrt)
            ta = pp.tile([P, 16], f32)
            ncA.vector.tensor_scalar(out=ta[:], in0=wbt[:, 0:16], scalar1=1e-30,
                                     scalar2=None, op0=ALU.add)
            ratio = pp.tile([P, 16], f32)
            ncA.vector.reciprocal(ratio[:], ta[:])
            ncA.vector.tensor_tensor(out=ratio[:], in0=wbt[:, 16:32], in1=ratio[:],
                                     op=ALU.mult)

            # slot chunks: load, scale by rsqrt(deg[row]), segmented-sum
            for ci, cgroups in enumerate(chunks):
                c0 = cgroups[0][3]                       # column offset of chunk
                ccols = sum((e - s) * w for s, e, w, _ in cgroups)
                xt = pool.tile([P, 2 * ccols], bf16, tag="x01")
                ncA.sync.dma_start(xt[:], x01s.ap()[:, 2 * c0:2 * (c0 + ccols)])
                dt = pool.tile([P, ccols], bf16, tag="dg")
                ncA.scalar.dma_start(dt[:], dgs.ap()[:, c0:c0 + ccols])
                rt = pool.tile([P, ccols], bf16, tag="r")
                for (l0, l1, w, off) in cgroups:
                    if w == 0:
                        continue
                    sz = (l1 - l0) * w
                    o = off - c0                         # local col offset in chunk
                    ncA.scalar.activation(rt[:, o:o + sz], dt[:, o:o + sz], AF.Abs_reciprocal_sqrt)
                    ncA.vector.tensor_tensor(
                        out=xt[:, 2 * o:2 * o + sz],
                        in0=xt[:, 2 * o:2 * o + sz], in1=rt[:, o:o + sz], op=ALU.mult)
                    ncA.vector.tensor_tensor(
                        out=xt[:, 2 * o + sz:2 * o + 2 * sz],
                        in0=xt[:, 2 * o + sz:2 * o + 2 * sz], in1=rt[:, o:o + sz],
                        op=ALU.mult)
                    with ncA.allow_low_precision(reason="bf16 agg, e2e-checked"):
                        ncA.vector.tensor_reduce(
                            out=agg[:].rearrange("p (f n) -> p f n", f=2)[:, :, l0:l1],
                            in_=xt[:, 2 * o:2 * o + 2 * sz].rearrange(
                                "p (f n w) -> p f n w", f=2, w=w),
                            axis=X, op=ALU.add)

            # memset agg columns not covered by any group (w == 0)
            covered = np.zeros(NSL, bool)
            for (l0, l1, w, _) in groups:
                if w > 0:
                    covered[l0:l1] = True
            run = None
            for l in range(NSL + 1):
                if l < NSL and not covered[l]:
                    run = l if run is None else run
                elif run is not None:
                    for half in range(2):
                        ncA.gpsimd.memset(agg[:, half * 256 + run:half * 256 + l], 0.0)
                    run = None

            # z = dinv * (agg + dinv * x), then the 2->16->1 MLP.
            # Processed in two column-range pieces: the first piece only needs
            # the agg columns of the first two DMA chunks, so it overlaps the
            # last chunk's transfer + reduces instead of waiting for them.
            z = pp.tile([P, 512], bf16)
            U = pp.tile([P, 16 * NSL], bf16)
            H = pp.tile([P, 16 * NSL], bf16)
            accA = pp.tile([P, NSL], f32)
            accB = pp.tile([P, NSL], f32)
            q = pp.tile([P, NSL], f32)
            split = chunks[1][-1][1] if len(chunks) >= 3 else NSL
            pieces = [(0, split)] + ([(split, NSL)] if split < NSL else [])
            for (lp0, lp1) in pieces:
                L = lp1 - lp0
                zv = z[:].rearrange("p (f n) -> p f n", f=2)[:, :, lp0:lp1]
                xv = tbt[:, 0:512].rearrange("p (f n) -> p f n", f=2)[:, :, lp0:lp1]
                agv = agg[:].rearrange("p (f n) -> p f n", f=2)[:, :, lp0:lp1]
                dv = dinv[:, lp0:lp1].rearrange(
                    "p (a n) -> p a n", a=1).to_broadcast([P, 2, L])
                ncA.vector.tensor_tensor(out=zv, in0=xv, in1=dv, op=ALU.mult)
                ncA.vector.tensor_tensor(out=zv, in0=zv, in1=agv, op=ALU.add)
                ncA.vector.tensor_tensor(out=zv, in0=zv, in1=dv, op=ALU.mult)
                z0 = z[:, lp0:lp1]
                z1 = z[:, NSL + lp0:NSL + lp1]
                # u_j for all j first (DVE back to back), relus pipeline on the
                # scalar engine behind them, then two independent acc chains.
                for j in range(16):
                    ncA.vector.scalar_tensor_tensor(
                        out=U[:, j * NSL + lp0:j * NSL + lp1], in0=z1,
                        scalar=ratio[:, j:j + 1], in1=z0,
                        op0=ALU.mult, op1=ALU.add)
                for j in range(16):
                    ncA.scalar.activation(H[:, j * NSL + lp0:j * NSL + lp1],
                                          U[:, j * NSL + lp0:j * NSL + lp1],
                                          AF.Relu,
                                          bias=wbt[:, 32 + j:33 + j],
                                          scale=ta[:, j:j + 1])
                for j in range(16):
                    dst = accA if j % 2 == 0 else accB
                    hj = H[:, j * NSL + lp0:j * NSL + lp1]
                    if j < 2:
                        ncA.vector.tensor_scalar(out=dst[:, lp0:lp1], in0=hj,
                                                 scalar1=wbt[:, 48 + j:49 + j],
                                                 scalar2=None, op0=ALU.mult)
                    else:
                        ncA.vector.scalar_tensor_tensor(
                            out=dst[:, lp0:lp1], in0=hj,
                            scalar=wbt[:, 48 + j:49 + j],
                            in1=dst[:, lp0:lp1], op0=ALU.mult, op1=ALU.add)
                ncA.vector.tensor_tensor(out=accA[:, lp0:lp1],
                                         in0=accA[:, lp0:lp1],
                                         in1=accB[:, lp0:lp1], op=ALU.add)
                # q = dinv * (h @ W2)
                ncA.vector.tensor_tensor(out=q[:, lp0:lp1],
                                         in0=accA[:, lp0:lp1],
                                         in1=dinv[:, lp0:lp1], op=ALU.mult)
            ncA.sync.dma_start(sOut.ap(), q[:])
    ncA.compile()

    # ---------------- program B ----------------
    ncB = bacc.Bacc("TRN2", target_bir_lowering=False, debug=False, num_devices=N_CORES)
    sgs = ncB.dram_tensor("sgs", [P, TOT], bf16, kind="ExternalInput")
    dgtB = ncB.dram_tensor("dgtB", [P, NSL], bf16, kind="ExternalInput")
    tf = ncB.dram_tensor("tf", [P, NSL + 1], f32, kind="ExternalInput")  # qt | b2
    out = ncB.dram_tensor("out", [P, NSL], f32, kind="ExternalOutput")

    with tile.TileContext(ncB) as tc:
        with tc.tile_pool(name="slots", bufs=2) as pool, \
             tc.tile_pool(name="persist", bufs=1) as pp:
            tft = pp.tile([P, NSL + 1], f32)
            ncB.scalar.dma_start(tft[:], tf.ap())
            dt = pp.tile([P, NSL], bf16)
            ncB.scalar.dma_start(dt[:], dgtB.ap())
            aggS = pp.tile([P, NSL], bf16)
            for ci, cgroups in enumerate(chunks):
                c0 = cgroups[0][3]
                ccols = sum((e - s) * w for s, e, w, _ in cgroups)
                st = pool.tile([P, ccols], bf16, tag="s")
                ncB.sync.dma_start(st[:], sgs.ap()[:, c0:c0 + ccols])
                for (l0, l1, w, off) in cgroups:
                    if w == 0:
                        continue
                    o = off - c0
                    with ncB.allow_low_precision(reason="bf16 agg, e2e-checked"):
                        ncB.vector.tensor_reduce(
                            out=aggS[:, l0:l1],
                            in_=st[:, o:o + (l1 - l0) * w].rearrange(
                                "p (n w) -> p n w", w=w),
                            axis=X, op=ALU.add)
            covered = np.zeros(NSL, bool)
            for (l0, l1, w, _) in groups:
                if w > 0:
                    covered[l0:l1] = True
            run = None
            for l in range(NSL + 1):
                if l < NSL and not covered[l]:
                    run = l if run is None else run
                elif run is not None:
                    ncB.gpsimd.memset(aggS[:, run:l], 0.0)
                    run = None
            dinv = pp.tile([P, NSL], bf16)
            ncB.scalar.activation(dinv[:], dt[:], AF.Abs_reciprocal_sqrt)
            o1 = pp.tile([P, NSL], f32)
            ncB.vector.tensor_tensor(out=o1[:], in0=aggS[:], in1=tft[:, 0:NSL],
                                     op=ALU.add)
            ncB.vector.tensor_tensor(out=o1[:], in0=o1[:], in1=dinv[:], op=ALU.mult)
            ncB.vector.tensor_scalar(out=o1[:], in0=o1[:],
                                     scalar1=tft[:, NSL:NSL + 1],
                                     scalar2=None, op0=ALU.add)
            ncB.sync.dma_start(out.ap(), o1[:])
    ncB.compile()
    return ncA, ncB


def kernel(x, edge_index, W1, b1, W2, b2, n_nodes):
    from concourse.bass_utils import run_bass_kernel_spmd

    N = int(n_nodes)
    x = np.asarray(x, dtype=np.float32)
    ei = np.asarray(edge_index)
    row = ei[0].astype(np.int64)
    col = ei[1].astype(np.int64)
    W1 = np.asarray(W1, np.float32); b1 = np.asarray(b1, np.float32)
    W2 = np.asarray(W2, np.float32); b2 = np.asarray(b2, np.float32)
    E = row.shape[0]

    # ---- host index layout (structural only) ----
    deg = np.bincount(col, minlength=N) + 1           # includes self-loop
    indeg = deg - 1
    order = np.argsort(-deg, kind="stable")           # rank -> node
    rank_of = np.empty(N, np.int64)
    rank_of[order] = np.arange(N)

    indeg_byrank = np.zeros(NRANKS, np.int64)
    indeg_byrank[:N] = indeg[order]
    W_l = indeg_byrank[np.arange(NSL) * (N_CORES * P)]   # local-slice width
    groups, TOT = _plan_groups(W_l)

    key = (TOT, tuple(g[:3] for g in groups))
    if key not in _cache:
        if TRACE:
            _install_ntff_shim()
        _cache[key] = _build_programs(groups, TOT)
    ncA, ncB = _cache[key]

    # per-group lookup tables indexed by local slice
    l2w = np.zeros(NSL, np.int64)
    l2off = np.zeros(NSL, np.int64)   # dgs column of slot (le, k=0)
    l2sz = np.zeros(NSL, np.int64)    # group block size (cols)
    l2go = np.zeros(NSL, np.int64)    # group col offset
    for (l0, l1, w, off) in groups:
        l2w[l0:l1] = w
        l2go[l0:l1] = off
        l2sz[l0:l1] = (l1 - l0) * w
        l2off[l0:l1] = off + (np.arange(l0, l1) - l0) * w

    # ---- per-edge slot placement ----
    re = rank_of[col]
    pe = re & 127
    sl = re >> 7
    ce = sl % N_CORES
    le = sl // N_CORES
    sidx = np.argsort(re, kind="stable")
    re_s = re[sidx]
    runstart = np.empty(E, bool)
    runstart[0] = True
    np.not_equal(re_s[1:], re_s[:-1], out=runstart[1:])
    starts = np.flatnonzero(runstart)
    rid = np.cumsum(runstart) - 1
    slot = np.empty(E, np.int64)
    slot[sidx] = np.arange(E) - starts[rid]
    posd = l2off[le] + slot                    # column in dgs
    posx0 = l2go[le] * 2 + (l2off[le] - l2go[le]) + slot   # column in x01s (x0)
    xsz = l2sz[le]                             # x1 column = posx0 + xsz

    x01s = np.zeros((N_CORES, P, 2 * TOT), BF16)
    dgs = np.ones((N_CORES, P, TOT), BF16)
    core_masks = []
    for c in range(N_CORES):
        m = ce == c
        core_masks.append(m)
        x01s[c][pe[m], posx0[m]] = x[row[m], 0].astype(BF16)
        x01s[c][pe[m], posx0[m] + xsz[m]] = x[row[m], 1].astype(BF16)
        dgs[c][pe[m], posd[m]] = deg[row[m]].astype(BF16)

    # ---- node tables ----
    pgrid = np.arange(P)[:, None]
    lgrid = np.arange(NSL)[None, :]
    tbs = np.zeros((N_CORES, P, 768), BF16)
    nodes_c = []
    valid_c = []
    for c in range(N_CORES):
        ranks = (lgrid * N_CORES + c) * P + pgrid          # [P, NSL]
        valid = ranks < N
        nodes = order[np.minimum(ranks, N - 1)]
        nodes_c.append(nodes); valid_c.append(valid)
        tbs[c, :, 0:256] = np.where(valid, x[nodes, 0], 0.0).astype(BF16)
        tbs[c, :, 256:512] = np.where(valid, x[nodes, 1], 0.0).astype(BF16)
        tbs[c, :, 512:768] = np.where(valid, deg[nodes].astype(np.float32),
                                      1.0).astype(BF16)

    wb = np.zeros((P, 65), np.float32)
    wb[:, 0:16] = W1[0]; wb[:, 16:32] = W1[1]
    wb[:, 32:48] = b1
    wb[:, 48:64] = W2[:, 0]
    wb[:, 64] = b2[0]

    in_maps_A = [{"x01s": x01s[c], "dgs": dgs[c], "tb": tbs[c], "wb": wb}
                 for c in range(N_CORES)]
    resA = run_bass_kernel_spmd(ncA, in_maps_A, core_ids=list(range(N_CORES)),
                                trace=TRACE)

    # ---- q table, host re-shard into slots ----
    q = np.zeros(N, np.float32)
    for c in range(N_CORES):
        v = valid_c[c]
        q[nodes_c[c][v]] = resA.results[c]["sOut"][v]
    kernel._dbg = {"q": q, "resA": resA}

    sgs = np.zeros((N_CORES, P, TOT), BF16)
    tfs = np.zeros((N_CORES, P, NSL + 1), np.float32)
    qrow = q[row].astype(BF16)
    for c in range(N_CORES):
        m = core_masks[c]
        sgs[c][pe[m], posd[m]] = qrow[m]
        tfs[c, :, 0:NSL] = np.where(valid_c[c], q[nodes_c[c]], 0.0)
        tfs[c, :, NSL] = b2[0]

    in_maps_B = [{"sgs": sgs[c], "dgtB": tbs[c, :, 512:768], "tf": tfs[c]}
                 for c in range(N_CORES)]
    resB = run_bass_kernel_spmd(ncB, in_maps_B, core_ids=list(range(N_CORES)),
                                trace=TRACE)

    outv = np.zeros(N, np.float32)
    for c in range(N_CORES):
        v = valid_c[c]
        outv[nodes_c[c][v]] = resB.results[c]["out"][v]
    kernel._dbg.update({"sgs": sgs, "tfs": tfs, "resB": resB,
                        "nodes_c": nodes_c, "valid_c": valid_c,
                        "groups": groups, "TOT": TOT})

    kernel.last_exec_ns = (getattr(resA, "exec_time_ns", None) or 0) + \
                          (getattr(resB, "exec_time_ns", None) or 0)
    return outv[:, None]


# revision 18
# speedup vs baseline: 1.0990x; 1.0990x over previous
"""2-layer GCN (GCNConv -> ReLU -> GCNConv) on 8 TRN2 NeuronCores.

Strategy (sliced-ELLPACK, node sharding, bf16 slots):
  GCN algebra: out = D^-1/2 (A+I) D^-1/2 (relu(D^-1/2 (A+I) D^-1/2 x W1 + b1)) W2 + b2.
  Normalization is separable (norm_e = dinv[row]*dinv[col]) and aggregation
  linear, so each layer is: per-node scale -> unweighted neighbor-sum ->
  per-node scale -> dense matmul. Self-loop contributions are dense adds.

  Host (pure index layout, no model math): sort nodes by in-degree, pack each
  node's incoming edges into padded slot rows (sliced ELLPACK, slices of 128
  nodes, DP-chosen groups of slices sharing a width). Slots carry the *input*
  features x[row] (bf16) and the integer structural count deg[row] (bf16,
  exact). Device computes all FP model math: rsqrt on the scalar engine,
  per-slot scaling + segmented reductions on DVE (bf16 2x mode), the
  2->16->1 MLP as a scalar_tensor_tensor chain with the per-j feature scale
  folded into the scalar-engine relu via a device-computed b/a ratio.

  Two launches: A computes layer 1 + q = dinv * (h @ W2); host re-shards q
  into the same slot layout (index gather only); B reduces q-slots and
  finishes layer 2. Nodes sharded round-robin by 128-node slice; edge slots
  live with their target node.
"""
import numpy as np
import ml_dtypes

BF16 = ml_dtypes.bfloat16
P = 128
N_CORES = 8
NSL = 256            # local slices per core -> 2048 global slices
NRANKS = 2048 * P    # padded rank space (262144 >= 250000)
LAM = 60             # DP penalty (columns) per extra width-group
N_CHUNKS = 3         # slot-array DMA chunks

TRACE = False
_cache = {}


def _install_ntff_shim():
    import contextlib, ctypes, sys, types
    if "antenv.axon_hooks" in sys.modules:
        return
    try:
        lib = ctypes.CDLL("/opt/axon/libaxon_pjrt.so")
        if not hasattr(lib, "axon_start_nrt_profile"):
            return
        lib.axon_start_nrt_profile.argtypes = [ctypes.POINTER(ctypes.c_int64), ctypes.c_size_t]
        lib.axon_start_nrt_profile.restype = ctypes.c_int64
        lib.axon_stop_nrt_profile.argtypes = [ctypes.c_char_p]
        lib.axon_stop_nrt_profile.restype = ctypes.c_int64
    except OSError:
        return

    @contextlib.contextmanager
    def _hook(output_dir, device_ids):
        import jax
        jax.devices()
        if device_ids:
            ids = (ctypes.c_int64 * len(device_ids))(*device_ids)
            rc = lib.axon_start_nrt_profile(ids, len(device_ids))
        else:
            rc = lib.axon_start_nrt_profile(None, 0)
        if rc != 0:
            raise RuntimeError(f"axon_start_nrt_profile rc={rc}")
        try:
            yield
        finally:
            lib.axon_stop_nrt_profile(str(output_dir).encode())

    mod = types.ModuleType("antenv.axon_hooks")
    mod.get_axon_ntff_profile_hook = lambda: _hook
    mod.set_axon_ntff_profile_hook = lambda h: None
    sys.modules["antenv.axon_hooks"] = mod


def _plan_groups(W_l):
    """DP partition of local slices into contiguous width-groups.
    Boundaries restricted to even slice indices so every group block has an
    even column count (keeps bf16 sub-blocks 4B-aligned for DVE 2x mode)."""
    nsl = len(W_l)
    INF = 1 << 60
    best = np.full(nsl + 1, INF, np.int64)
    best[0] = 0
    ch = np.zeros(nsl + 1, np.int64)
    for e in range(2, nsl + 1, 2):
        s = np.arange(0, e, 2)
        c = best[s] + (e - s) * W_l[s] + LAM
        i = int(np.argmin(c))
        best[e] = c[i]
        ch[e] = 2 * i
    bnds = []
    e = nsl
    while e > 0:
        s = int(ch[e])
        bnds.append((s, e))
        e = s
    bnds.reverse()
    groups = []
    off = 0
    for s, e in bnds:
        w = int(W_l[s])
        groups.append((s, e, w, off))   # (l0, l1, width, column offset in dgs)
        off += (e - s) * w
    return groups, off                  # TOT = off


def _chunk_groups(groups):
    """Split groups into DMA chunks with graded sizes (small first chunk so
    compute starts as early as possible while later transfers overlap)."""
    tot = sum((e - s) * w for s, e, w, _ in groups)
    fracs = [0.22, 0.58, 1.01]
    chunks = []
    cur = []
    acc = 0
    for g in groups:
        s, e, w, _ = g
        cur.append(g)
        acc += (e - s) * w
        if len(chunks) < len(fracs) - 1 and acc >= fracs[len(chunks)] * tot:
            chunks.append(cur)
            cur = []
    if cur:
        chunks.append(cur)
    return [c for c in chunks if c]


def _build_programs(groups, TOT):
    import concourse.bass as bass
    import concourse.bacc as bacc
    import concourse.tile as tile
    import concourse.mybir as mybir

    f32 = mybir.dt.float32
    bf16 = mybir.dt.bfloat16
    AF = mybir.ActivationFunctionType
    ALU = mybir.AluOpType
    X = mybir.AxisListType.X
    chunks = _chunk_groups(groups)

    # ---------------- program A ----------------
    ncA = bacc.Bacc("TRN2", target_bir_lowering=False, debug=False, num_devices=N_CORES)
    x01s = ncA.dram_tensor("x01s", [P, 2 * TOT], bf16, kind="ExternalInput")
    dgs = ncA.dram_tensor("dgs", [P, TOT], bf16, kind="ExternalInput")
    tb = ncA.dram_tensor("tb", [P, 768], bf16, kind="ExternalInput")    # xt01 | dgt
    wb = ncA.dram_tensor("wb", [P, 65], f32, kind="ExternalInput")
    sOut = ncA.dram_tensor("sOut", [P, NSL], f32, kind="ExternalOutput")

    with tile.TileContext(ncA) as tc:
        with tc.tile_pool(name="slots", bufs=2) as pool, \
             tc.tile_pool(name="persist", bufs=1) as pp:
            tbt = pp.tile([P, 768], bf16)
            ncA.scalar.dma_start(tbt[:], tb.ap())
            wbt = pp.tile([P, 65], f32)
            ncA.scalar.dma_start(wbt[:], wb.ap())
            agg = pp.tile([P, 512], bf16)

            # node-table prep: dinv, dinv^2, weight ratios
            dinv = pp.tile([P, NSL], bf16)
            ncA.scalar.activation(dinv[:], tbt[:, 512:768], AF.Abs_reciprocal_sqrt)
            ta = pp.tile([P, 16], f32)
            ncA.vector.tensor_scalar(out=ta[:], in0=wbt[:, 0:16], scalar1=1e-30,
                                     scalar2=None, op0=ALU.add)
            ratio = pp.tile([P, 16], f32)
            ncA.vector.reciprocal(ratio[:], ta[:])
            ncA.vector.tensor_tensor(out=ratio[:], in0=wbt[:, 16:32], in1=ratio[:],
                                     op=ALU.mult)

            # slot chunks: load, scale by rsqrt(deg[row]), segmented-sum
            for ci, cgroups in enumerate(chunks):
                c0 = cgroups[0][3]                       # column offset of chunk
                ccols = sum((e - s) * w for s, e, w, _ in cgroups)
                xt = pool.tile([P, 2 * ccols], bf16, tag="x01")
                ncA.sync.dma_start(xt[:], x01s.ap()[:, 2 * c0:2 * (c0 + ccols)])
                dt = pool.tile([P, ccols], bf16, tag="dg")
                ncA.scalar.dma_start(dt[:], dgs.ap()[:, c0:c0 + ccols])
                rt = pool.tile([P, ccols], bf16, tag="r")
                for (l0, l1, w, off) in cgroups:
                    if w == 0:
                        continue
                    sz = (l1 - l0) * w
                    o = off - c0                         # local col offset in chunk
                    ncA.scalar.activation(rt[:, o:o + sz], dt[:, o:o + sz], AF.Abs_reciprocal_sqrt)
                    ncA.vector.tensor_tensor(
                        out=xt[:, 2 * o:2 * o + sz],
                        in0=xt[:, 2 * o:2 * o + sz], in1=rt[:, o:o + sz], op=ALU.mult)
                    ncA.vector.tensor_tensor(
                        out=xt[:, 2 * o + sz:2 * o + 2 * sz],
                        in0=xt[:, 2 * o + sz:2 * o + 2 * sz], in1=rt[:, o:o + sz],
                        op=ALU.mult)
                    with ncA.allow_low_precision(reason="bf16 agg, e2e-checked"):
                        ncA.vector.tensor_reduce(
                            out=agg[:].rearrange("p (f n) -> p f n", f=2)[:, :, l0:l1],
                            in_=xt[:, 2 * o:2 * o + 2 * sz].rearrange(
                                "p (f n w) -> p f n w", f=2, w=w),
                            axis=X, op=ALU.add)

            # memset agg columns not covered by any group (w == 0)
            covered = np.zeros(NSL, bool)
            for (l0, l1, w, _) in groups:
                if w > 0:
                    covered[l0:l1] = True
            run = None
            for l in range(NSL + 1):
                if l < NSL and not covered[l]:
                    run = l if run is None else run
                elif run is not None:
                    for half in range(2):
                        ncA.gpsimd.memset(agg[:, half * 256 + run:half * 256 + l], 0.0)
                    run = None

            # z = dinv * (agg + dinv * x)   (both features in one [P,512] tile)
            z = pp.tile([P, 512], bf16)
            ncA.vector.tensor_tensor(
                out=z[:].rearrange("p (f n) -> p f n", f=2),
                in0=tbt[:, 0:512].rearrange("p (f n) -> p f n", f=2),
                in1=dinv[:].rearrange("p (a n) -> p a n", a=1).to_broadcast([P, 2, NSL]),
                op=ALU.mult)
            ncA.vector.tensor_tensor(out=z[:], in0=z[:], in1=agg[:], op=ALU.add)
            ncA.vector.tensor_tensor(
                out=z[:].rearrange("p (f n) -> p f n", f=2),
                in0=z[:].rearrange("p (f n) -> p f n", f=2),
                in1=dinv[:].rearrange("p (a n) -> p a n", a=1).to_broadcast([P, 2, NSL]),
                op=ALU.mult)

            # h_j = relu(a_j z0 + b_j z1 + c_j) = relu(a'_j * (z0 + ratio_j z1) + c_j)
            # acc = sum_j w2_j h_j. All u_j emitted first (DVE runs them back to
            # back), relus pipeline on the scalar engine behind them, then two
            # independent accumulation chains so acc never stalls on a fresh relu.
            z0 = z[:, 0:NSL]
            z1 = z[:, NSL:512]
            U = pp.tile([P, 16 * NSL], bf16)
            H = pp.tile([P, 16 * NSL], bf16)
            accA = pp.tile([P, NSL], f32)
            accB = pp.tile([P, NSL], f32)
            q = pp.tile([P, NSL], f32)
            for j in range(16):
                ncA.vector.scalar_tensor_tensor(
                    out=U[:, j * NSL:(j + 1) * NSL], in0=z1,
                    scalar=ratio[:, j:j + 1], in1=z0,
                    op0=ALU.mult, op1=ALU.add)
            for j in range(16):
                ncA.scalar.activation(H[:, j * NSL:(j + 1) * NSL],
                                      U[:, j * NSL:(j + 1) * NSL], AF.Relu,
                                      bias=wbt[:, 32 + j:33 + j],
                                      scale=ta[:, j:j + 1])
            for j in range(16):
                dst = accA if j % 2 == 0 else accB
                hj = H[:, j * NSL:(j + 1) * NSL]
                if j < 2:
                    ncA.vector.tensor_scalar(out=dst[:], in0=hj,
                                             scalar1=wbt[:, 48 + j:49 + j],
                                             scalar2=None, op0=ALU.mult)
                else:
                    ncA.vector.scalar_tensor_tensor(
                        out=dst[:], in0=hj, scalar=wbt[:, 48 + j:49 + j],
                        in1=dst[:], op0=ALU.mult, op1=ALU.add)
            ncA.vector.tensor_tensor(out=accA[:], in0=accA[:], in1=accB[:],
                                     op=ALU.add)
            # q = dinv * (h @ W2)
            ncA.vector.tensor_tensor(out=q[:], in0=accA[:], in1=dinv[:], op=ALU.mult)
            ncA.sync.dma_start(sOut.ap(), q[:])
    ncA.compile()

    # ---------------- program B ----------------
    ncB = bacc.Bacc("TRN2", target_bir_lowering=False, debug=False, num_devices=N_CORES)
    sgs = ncB.dram_tensor("sgs", [P, TOT], bf16, kind="ExternalInput")
    dgtB = ncB.dram_tensor("dgtB", [P, NSL], bf16, kind="ExternalInput")
    tf = ncB.dram_tensor("tf", [P, NSL + 1], f32, kind="ExternalInput")  # qt | b2
    out = ncB.dram_tensor("out", [P, NSL], f32, kind="ExternalOutput")

    with tile.TileContext(ncB) as tc:
        with tc.tile_pool(name="slots", bufs=3) as pool, \
             tc.tile_pool(name="persist", bufs=1) as pp:
            tft = pp.tile([P, NSL + 1], f32)
            ncB.scalar.dma_start(tft[:], tf.ap())
            dt = pp.tile([P, NSL], bf16)
            ncB.scalar.dma_start(dt[:], dgtB.ap())
            aggS = pp.tile([P, NSL], bf16)
            for ci, cgroups in enumerate(chunks):
                c0 = cgroups[0][3]
                ccols = sum((e - s) * w for s, e, w, _ in cgroups)
                st = pool.tile([P, ccols], bf16, tag="s")
                ncB.sync.dma_start(st[:], sgs.ap()[:, c0:c0 + ccols])
                for (l0, l1, w, off) in cgroups:
                    if w == 0:
                        continue
                    o = off - c0
                    with ncB.allow_low_precision(reason="bf16 agg, e2e-checked"):
                        ncB.vector.tensor_reduce(
                            out=aggS[:, l0:l1],
                            in_=st[:, o:o + (l1 - l0) * w].rearrange(
                                "p (n w) -> p n w", w=w),
                            axis=X, op=ALU.add)
            covered = np.zeros(NSL, bool)
            for (l0, l1, w, _) in groups:
                if w > 0:
                    covered[l0:l1] = True
            run = None
            for l in range(NSL + 1):
                if l < NSL and not covered[l]:
                    run = l if run is None else run
                elif run is not None:
                    ncB.gpsimd.memset(aggS[:, run:l], 0.0)
                    run = None
            dinv = pp.tile([P, NSL], bf16)
            ncB.scalar.activation(dinv[:], dt[:], AF.Abs_reciprocal_sqrt)
            o1 = pp.tile([P, NSL], f32)
            ncB.vector.tensor_tensor(out=o1[:], in0=aggS[:], in1=tft[:, 0:NSL],
                                     op=ALU.add)
            ncB.vector.tensor_tensor(out=o1[:], in0=o1[:], in1=dinv[:], op=ALU.mult)
            ncB.vector.tensor_scalar(out=o1[:], in0=o1[:],
                                     scalar1=tft[:, NSL:NSL + 1],
                                     scalar2=None, op0=ALU.add)
            ncB.sync.dma_start(out.ap(), o1[:])
    ncB.compile()
    return ncA, ncB


def kernel(x, edge_index, W1, b1, W2, b2, n_nodes):
    from concourse.bass_utils import run_bass_kernel_spmd

    N = int(n_nodes)
    x = np.asarray(x, dtype=np.float32)
    ei = np.asarray(edge_index)
    row = ei[0].astype(np.int64)
    col = ei[1].astype(np.int64)
    W1 = np.asarray(W1, np.float32); b1 = np.asarray(b1, np.float32)
    W2 = np.asarray(W2, np.float32); b2 = np.asarray(b2, np.float32)
    E = row.shape[0]

    # ---- host index layout (structural only) ----
    deg = np.bincount(col, minlength=N) + 1           # includes self-loop
    indeg = deg - 1
    order = np.argsort(-deg, kind="stable")           # rank -> node
    rank_of = np.empty(N, np.int64)
    rank_of[order] = np.arange(N)

    indeg_byrank = np.zeros(NRANKS, np.int64)
    indeg_byrank[:N] = indeg[order]
    W_l = indeg_byrank[np.arange(NSL) * (N_CORES * P)]   # local-slice width
    groups, TOT = _plan_groups(W_l)

    key = (TOT, tuple(g[:3] for g in groups))
    if key not in _cache:
        if TRACE:
            _install_ntff_shim()
        _cache[key] = _build_programs(groups, TOT)
    ncA, ncB = _cache[key]

    # per-group lookup tables indexed by local slice
    l2w = np.zeros(NSL, np.int64)
    l2off = np.zeros(NSL, np.int64)   # dgs column of slot (le, k=0)
    l2sz = np.zeros(NSL, np.int64)    # group block size (cols)
    l2go = np.zeros(NSL, np.int64)    # group col offset
    for (l0, l1, w, off) in groups:
        l2w[l0:l1] = w
        l2go[l0:l1] = off
        l2sz[l0:l1] = (l1 - l0) * w
        l2off[l0:l1] = off + (np.arange(l0, l1) - l0) * w

    # ---- per-edge slot placement ----
    re = rank_of[col]
    pe = re & 127
    sl = re >> 7
    ce = sl % N_CORES
    le = sl // N_CORES
    sidx = np.argsort(re, kind="stable")
    re_s = re[sidx]
    runstart = np.empty(E, bool)
    runstart[0] = True
    np.not_equal(re_s[1:], re_s[:-1], out=runstart[1:])
    starts = np.flatnonzero(runstart)
    rid = np.cumsum(runstart) - 1
    slot = np.empty(E, np.int64)
    slot[sidx] = np.arange(E) - starts[rid]
    posd = l2off[le] + slot                    # column in dgs
    posx0 = l2go[le] * 2 + (l2off[le] - l2go[le]) + slot   # column in x01s (x0)
    xsz = l2sz[le]                             # x1 column = posx0 + xsz

    x01s = np.zeros((N_CORES, P, 2 * TOT), BF16)
    dgs = np.ones((N_CORES, P, TOT), BF16)
    core_masks = []
    for c in range(N_CORES):
        m = ce == c
        core_masks.append(m)
        x01s[c][pe[m], posx0[m]] = x[row[m], 0].astype(BF16)
        x01s[c][pe[m], posx0[m] + xsz[m]] = x[row[m], 1].astype(BF16)
        dgs[c][pe[m], posd[m]] = deg[row[m]].astype(BF16)

    # ---- node tables ----
    pgrid = np.arange(P)[:, None]
    lgrid = np.arange(NSL)[None, :]
    tbs = np.zeros((N_CORES, P, 768), BF16)
    nodes_c = []
    valid_c = []
    for c in range(N_CORES):
        ranks = (lgrid * N_CORES + c) * P + pgrid          # [P, NSL]
        valid = ranks < N
        nodes = order[np.minimum(ranks, N - 1)]
        nodes_c.append(nodes); valid_c.append(valid)
        tbs[c, :, 0:256] = np.where(valid, x[nodes, 0], 0.0).astype(BF16)
        tbs[c, :, 256:512] = np.where(valid, x[nodes, 1], 0.0).astype(BF16)
        tbs[c, :, 512:768] = np.where(valid, deg[nodes].astype(np.float32),
                                      1.0).astype(BF16)

    wb = np.zeros((P, 65), np.float32)
    wb[:, 0:16] = W1[0]; wb[:, 16:32] = W1[1]
    wb[:, 32:48] = b1
    wb[:, 48:64] = W2[:, 0]
    wb[:, 64] = b2[0]

    in_maps_A = [{"x01s": x01s[c], "dgs": dgs[c], "tb": tbs[c], "wb": wb}
                 for c in range(N_CORES)]
    resA = run_bass_kernel_spmd(ncA, in_maps_A, core_ids=list(range(N_CORES)),
                                trace=TRACE)

    # ---- q table, host re-shard into slots ----
    q = np.zeros(N, np.float32)
    for c in range(N_CORES):
        v = valid_c[c]
        q[nodes_c[c][v]] = resA.results[c]["sOut"][v]
    kernel._dbg = {"q": q, "resA": resA}

    sgs = np.zeros((N_CORES, P, TOT), BF16)
    tfs = np.zeros((N_CORES, P, NSL + 1), np.float32)
    qrow = q[row].astype(BF16)
    for c in range(N_CORES):
        m = core_masks[c]
        sgs[c][pe[m], posd[m]] = qrow[m]
        tfs[c, :, 0:NSL] = np.where(valid_c[c], q[nodes_c[c]], 0.0)
        tfs[c, :, NSL] = b2[0]

    in_maps_B = [{"sgs": sgs[c], "dgtB": tbs[c, :, 512:768], "tf": tfs[c]}
                 for c in range(N_CORES)]
    resB = run_bass_kernel_spmd(ncB, in_maps_B, core_ids=list(range(N_CORES)),
                                trace=TRACE)

    outv = np.zeros(N, np.float32)
    for c in range(N_CORES):
        v = valid_c[c]
        outv[nodes_c[c][v]] = resB.results[c]["out"][v]
    kernel._dbg.update({"sgs": sgs, "tfs": tfs, "resB": resB,
                        "nodes_c": nodes_c, "valid_c": valid_c,
                        "groups": groups, "TOT": TOT})

    kernel.last_exec_ns = (getattr(resA, "exec_time_ns", None) or 0) + \
                          (getattr(resB, "exec_time_ns", None) or 0)
    return outv[:, None]
